# revision 17
# baseline (speedup 1.0000x reference)
"""Trainium2 Bass kernel for ContinuousAxialDW.

The reference op (continuous-offset axial depthwise conv, bilinear sampling)
collapses to two 1D depthwise convolutions with *integer* shifts, because the
bilinear fraction frac(off*r) is constant along the sampled axis:

    out[b,c,h,w] = x + sum_s A[c,s]*x[b,c,h+s,w] + sum_t B[c,t]*x[b,c,h,w+t]

with zero padding at the borders.  The two conv terms are per-channel banded
matmuls (Mh @ X and X @ Sw with 256x256 banded Mh/Sw); the identity term is
added on the host in exact fp32 during the unshard, so the device only
computes the *small* residual terms and can run entirely in fp8:

  * All matmuls use fp8e4 (e4m3) operands in DoubleRow perf modes, which
    contract 2 k-tiles (256 rows) per instruction at 0.5 cyc/row.
  * term1 = Mh @ X: k-tiles are the two 128-row h blocks of X (natural
    layout, plain DoubleRow), one matmul per (output h block, image pair).
  * term2 = X @ Sw: contracts w, so it consumes a host-pre-transposed copy
    of x in fp8 *pair* layout (partition p holds w = 2p and 2p+1): the
    k-tiles are the even/odd w lanes.  The stationary operand is stored in
    DoubleRowSwInterleave's native layout (pairs interleaved per column,
    columns reversed) so its ldweights AP is plain contiguous.
  * Both terms accumulate in fp32 PSUM (2 banks per image pair), evacuated
    once per pair by DVE/ACT (alternating) with a per-channel scale into
    int8.  The scale 127/bound_c (bound_c = sum|coeffs| * max|x|, an exact
    bound computed on host) makes int8 quantization ~5x more accurate than
    fp8e3 for these narrow-range residuals, at the same DMA cost.

Sharding: channels across the 8 cores (12 ch/core, all 8 batch images).
DMA per core: x fp8 6.3MB + x^T fp8 6.3MB + mats 1.6MB in, terms int8
6.3MB out; all layouts host-pre-shuffled so every DMA moves >=1KB
contiguous per partition.
"""

import os
import sys

import numpy as np
import ml_dtypes

for _p in ("/opt/trn_rl_repo", "/root/.axon_site/_ro/trn_rl_repo"):
    if _p not in sys.path and os.path.isdir(_p):
        sys.path.append(_p)

import concourse.bass as bass
import concourse.mybir as mybir
from concourse import bacc, tile
from concourse.bass_utils import run_bass_kernel_spmd

N_CORES = 8
B, C, H, W = 8, 96, 256, 256
C_LOC = C // N_CORES  # 12 channels per core

F32 = mybir.dt.float32
F8 = mybir.dt.float8e4
I8 = mybir.dt.int8
NP8 = ml_dtypes.float8_e4m3

LAST_RESULTS = None
_PROGRAM = None

DR = mybir.MatmulPerfMode.DoubleRow
DRSI = mybir.MatmulPerfMode.DoubleRowSwInterleave


def _emit(tc, x_d, xt_d, m_d, s_d, o_d):
    """Per-core program.

    DRAM (per core):
      x_d:  [C_LOC, 128, 2, 8, 256] f8e4   x_d[c,p,i,b,w] = x[b,cg,128i+p,w]
      xt_d: [C_LOC, 128, 8, 2, 256] f8e4   xt_d[c,p,b,i,2k+e]
                                             = x[b,cg,128i+(127-k),2p+e]
      m_d:  [C_LOC, 128, 2, 2, 256] f8e4   [.,p,0,i,n] = Mh[n,128i+p] (banded)
                                           [.,p,1,e,n] = Sw[2p+e,n]
      s_d:  [128, C_LOC] f32               127/bound_c broadcast down partitions
      o_d:  [C_LOC, 128, 2, 8, 256] int8   residual terms * 127/bound_c
    """
    nc = tc.nc
    n_pairs = 4 * C_LOC
    with (
        tc.tile_pool(name="const", bufs=1) as cpool,
        tc.tile_pool(name="mats", bufs=3) as mpool,
        tc.tile_pool(name="xin", bufs=3) as xpool,
        tc.tile_pool(name="xtin", bufs=3) as xtpool,
        tc.tile_pool(name="outp", bufs=3) as opool,
        tc.tile_pool(name="pso", bufs=2, space="PSUM") as pso,
    ):
        sc_t = cpool.tile([128, C_LOC], F32, name="scales")
        nc.sync.dma_start(sc_t[:], s_d[:])

        chans = {}  # c -> (x_t, xt_t, mat_t, o_t)

        def start_channel(c):
            mat_t = mpool.tile([128, 2, 2, 256], F8, name=f"m{c}", tag="m")
            nc.sync.dma_start(mat_t[:], m_d[c])
            x_t = xpool.tile([128, 2, 8, 256], F8, name=f"x{c}", tag="x")
            xt_t = xtpool.tile([128, 8, 2, 256], F8, name=f"xt{c}", tag="xt")
            if c == 0:
                # split the first channel's loads so group 0 starts ASAP
                nc.sync.dma_start(x_t[:, :, 0:4, :], x_d[c, :, :, 0:4, :])
                nc.scalar.dma_start(xt_t[:, 0:4], xt_d[c, :, 0:4])
                nc.sync.dma_start(x_t[:, :, 4:8, :], x_d[c, :, :, 4:8, :])
                nc.scalar.dma_start(xt_t[:, 4:8], xt_d[c, :, 4:8])
            else:
                nc.sync.dma_start(x_t[:], x_d[c])
                nc.scalar.dma_start(xt_t[:], xt_d[c])
            o_t = opool.tile([128, 2, 8, 256], I8, name=f"o{c}", tag="o")
            chans[c] = (x_t, xt_t, mat_t, o_t)

        def group(q):
            # one 4-image group: term1 batches all 4 images per h block
            # (N=1024, PSUM out spans 2 banks); term2 adds one DoubleRow
            # matmul per (h block, image).  10 matmuls per group.
            c, ql = divmod(q, 2)
            b0 = 4 * ql
            x_t, xt_t, mat_t, o_t = chans[c]
            po = pso.tile([128, 2, 1024], F32, name=f"po{q}", tag="po")
            for hb in range(2):
                # each 512-col half of po[:, hb] is its own PSUM bank: arm
                # (start) on its first matmul, close (stop) on its last.
                for half in range(2):
                    nc.tensor.matmul(
                        po[:, hb, half * 512 : half * 512 + 512],
                        lhsT=mat_t[:, 0, :, hb * 128 : hb * 128 + 128],
                        rhs=x_t[:, :, b0 + 2 * half : b0 + 2 * half + 2, :],
                        start=True,
                        stop=False,
                        perf_mode=DR,
                        skip_group_check=True,
                    )
                for bl in range(4):
                    nc.tensor.matmul(
                        po[:, hb, bl * 256 : bl * 256 + 256],
                        lhsT=xt_t[:, b0 + bl, hb, :],
                        rhs=mat_t[:, 1, :, :],
                        start=False,
                        stop=(bl % 2 == 1),
                        perf_mode=DRSI,
                        skip_group_check=True,
                    )
            # evacuate the two PSUM halves on both engines in parallel to
            # halve the PSUM->SBUF handoff latency (PE is gated on po reuse)
            dst = o_t[:, :, b0 : b0 + 4, :]
            nc.scalar.activation(
                o_t[:, 0, b0 : b0 + 4, :], po[:, 0],
                mybir.ActivationFunctionType.Copy, scale=sc_t[:, c : c + 1],
            )
            nc.vector.tensor_scalar_mul(
                o_t[:, 1, b0 : b0 + 4, :], po[:, 1], sc_t[:, c : c + 1]
            )
            # store each finished half-channel immediately (keeps the tail short)
            nc.gpsimd.dma_start(o_d[c][:, :, b0 : b0 + 4, :], dst)

        for q in range(2 * C_LOC):
            c, ql = divmod(q, 2)
            if ql == 0:
                start_channel(c)
            group(q)


def _build_program():
    global _PROGRAM
    if _PROGRAM is not None:
        return _PROGRAM
    nc = bacc.Bacc("TRN2", target_bir_lowering=False, debug=False, num_devices=N_CORES)
    x_d = nc.dram_tensor("x_sh", [C_LOC, 128, 2, 8, 256], F8, kind="ExternalInput").ap()
    xt_d = nc.dram_tensor("xt_sh", [C_LOC, 128, 8, 2, 256], F8, kind="ExternalInput").ap()
    m_d = nc.dram_tensor("mats", [C_LOC, 128, 2, 2, 256], F8, kind="ExternalInput").ap()
    s_d = nc.dram_tensor("scales", [128, C_LOC], F32, kind="ExternalInput").ap()
    o_d = nc.dram_tensor("out_sh", [C_LOC, 128, 2, 8, 256], I8, kind="ExternalOutput").ap()
    with tile.TileContext(nc) as tc:
        _emit(tc, x_d, xt_d, m_d, s_d, o_d)
    nc.compile()
    _PROGRAM = nc
    return nc


def _eff_coeffs(taps, r):
    """taps: [k, C] per-tap depthwise weights -> dict integer_shift -> coeff[C].

    Mirrors the reference: pos = coord + off*r (f32), i0 = floor(pos),
    frac = pos - i0; both are constant per tap since coord is integral.
    """
    r_val = max(float(np.float32(r)), 1.0)
    k = taps.shape[0]
    pad = k // 2
    coeffs = {}
    for i, off in enumerate(range(-pad, pad + 1)):
        pos = np.float32(off * np.float32(r_val))
        s0 = int(np.floor(pos))
        f = float(np.float32(pos)) - s0
        for s, cmul in ((s0, 1.0 - f), (s0 + 1, f)):
            if cmul != 0.0:
                acc = coeffs.setdefault(s, np.zeros(taps.shape[1], np.float64))
                acc += cmul * taps[i].astype(np.float64)
    return coeffs


def _build_mats(weight_h, weight_w, r):
    """Banded matrices (no identity) in DoubleRow layout [C, 128, 2, 2, 256],
    plus a per-channel bound coefficient: max_row sum|Mh| + max_row sum|Sw|."""
    ch = _eff_coeffs(weight_h[:, 0, :, 0].T, r)
    cw = _eff_coeffs(weight_w[:, 0, 0, :].T, r)
    mh_t = np.zeros((C, H, H), np.float64)  # [c, h_in, h_out] = Mh[h_out, h_in]
    for s, coef in ch.items():
        i = np.arange(max(0, s), H + min(0, s))
        mh_t[:, i, i - s] += coef[:, None]
    sw = np.zeros((C, W, W), np.float64)  # [c, w_in, w_out]
    for t, coef in cw.items():
        i = np.arange(max(0, t), W + min(0, t))
        sw[:, i, i - t] += coef[:, None]
    mats = np.empty((C, 128, 2, 2, 256), np.float32)
    mats[:, :, 0] = mh_t.reshape(C, 2, 128, 256).transpose(0, 2, 1, 3)
    mats[:, :, 1] = sw.reshape(C, 128, 2, 256)
    mats8 = mats.astype(NP8)
    m8 = mats8.astype(np.float64)
    bound_h = np.abs(m8[:, :, 0]).sum(axis=(1, 2)).max(axis=1)
    bound_w = np.abs(m8[:, :, 1]).sum(axis=(1, 2)).max(axis=1)
    return mats8, np.maximum(bound_h + bound_w, 1e-6)


def kernel(**inputs):
    global LAST_RESULTS
    x = np.ascontiguousarray(np.asarray(inputs["x"], dtype=np.float32))
    weight_h = np.asarray(inputs["weight_h"], dtype=np.float32)
    weight_w = np.asarray(inputs["weight_w"], dtype=np.float32)
    r = np.asarray(inputs["r"], dtype=np.float32)
    assert x.shape == (B, C, H, W), x.shape

    mats, coef_bound = _build_mats(weight_h, weight_w, r)
    xq = x.astype(NP8)  # quantize once; both layouts share the same values
    xmax = float(np.abs(xq.astype(np.float32)).max())
    bound = coef_bound * xmax * 1.0001  # |terms| <= bound_c exactly
    scales = 127.0 / bound  # [C]

    # natural layout [C, 128(p), 2(i), B, W]
    xs = (
        xq.transpose(1, 2, 0, 3)
        .reshape(C, 2, 128, B, W)
        .transpose(0, 2, 1, 3, 4)
    )
    # pair-transposed SwInterleave layout [C, 128(p), B, 2(i), 256(2k+e)]
    xt = (
        xq.transpose(1, 3, 0, 2)  # [C, W, B, H]
        .reshape(C, 128, 2, B, 2, 128)[:, :, :, :, :, ::-1]  # reverse k
        .transpose(0, 1, 3, 4, 5, 2)  # [C, p, B, i, k, e]
        .reshape(C, 128, B, 2, 256)
    )

    nc = _build_program()
    in_maps = [
        {
            "x_sh": np.ascontiguousarray(xs[i * C_LOC : (i + 1) * C_LOC]),
            "xt_sh": np.ascontiguousarray(xt[i * C_LOC : (i + 1) * C_LOC]),
            "mats": np.ascontiguousarray(mats[i * C_LOC : (i + 1) * C_LOC]),
            "scales": np.ascontiguousarray(
                np.broadcast_to(
                    scales[i * C_LOC : (i + 1) * C_LOC].astype(np.float32),
                    (128, C_LOC),
                )
            ),
        }
        for i in range(N_CORES)
    ]
    res = run_bass_kernel_spmd(nc, in_maps, list(range(N_CORES)))
    LAST_RESULTS = res
    # [C_LOC, 128, 2, 8, 256] int8 per core -> terms [B, C, H, W]
    o = np.concatenate([res.results[i]["out_sh"] for i in range(N_CORES)], axis=0)
    deq = (bound / 127.0).astype(np.float32)[:, None, None, None, None]
    terms = o.astype(np.float32) * deq
    terms = (
        terms.transpose(0, 2, 1, 3, 4).reshape(C, 256, B, W).transpose(2, 0, 1, 3)
    )
    return x + terms
